# revision 47
# baseline (speedup 1.0000x reference)
"""BDHChess Trainium2 kernel — factored (rho-free) linear-attention formulation,
v3: transposed (feature-major) fp16 state, PE Gram dots, FM matmul orientation.

Self-contained: kernel(**inputs) takes FULL unsharded inputs (as in
reference.setup_inputs()) and returns the FULL (512, 1) float32 output.

Strategy
--------
Pure data-parallel over batch: 512 boards -> 8 NeuronCores x 64. Weights are
replicated; no collectives.

rho_0 = 0 and rho_{k+1} = lam*rho_k + (1-lam)*outer(xw_k, z_k), so

    a_k = sum_{m<=k} lam^k * [c_m * recip_m] * (x_m . x_k)_[b,h] * z_{m-1}[b,h,:]

with c_m = (1-lam)*lam^-m, recip_m = 1/||x_m||_head; rho never exists.  x_m
live ONLY in transposed "T-form" slots (xthist): slot s=4t+cc holds chunk
[n=128, (hh,b)=128] of head-pair t, so
  * the K^2/2 dot products run as PE Gram matmuls (4 x 53ns accumulating mms
    -> [128,128] Gram whose diagonal is extracted by one masked STT on DVE,
    with the rw scale + Sp accumulation fused),
  * W_dy / E / W_dx run in feature-major orientation (stationary = weight
    chunk, moving = 64-wide fp16 activations -> 1 cycle/row),
  * the self-Gram diagonal doubles as ||x||^2 for the norm chain, and
  * y = relu(g) * x and x += relu(dx) are single STTs straight from PSUM.

Layouts (per core, Bc=64):
  BM "batch-major": (64 part = b, features free) - LN stats/applies, z, tc.
  T-form:           (128 part = n-chunk, slot s=4t+cc, 64*hh+b) - x, g, y, dx.
  ZBH:              (128 part = 64*hh+b, 2 t, 64 d) - z-history, a-accum.
Partition moves (ZBH <-> BM) are identity-matmuls on the PE; no DMA inside
the step loop.  PSUM stays fp32; fp16 keeps rounding error ~1e-3.
"""

import numpy as np
from contextlib import ExitStack

B, D, NH, NN, DH, K = 512, 256, 4, 512, 64, 8
SIM_GELU_SUB = False            # CoreSim lacks Gelu; substitute Tanh for sim runs
NTOT = NH * NN                  # 2048
NCORES = 8
BC = B // NCORES                # 64
EPS_LN = 1e-5
EPS_NRM = 1e-8


def _prep_host(inputs):
    """Host-side preprocessing: flatten boards, fold LN affines into weights,
    compute lam, detect skippable (zero) biases."""
    f32 = np.float32
    ap = {k: np.ascontiguousarray(np.asarray(v), dtype=f32) for k, v in inputs.items()}

    lam = float(1.0 / (1.0 + np.exp(-ap["log_damping"].reshape(-1)[0])))

    boards = ap["boards"].reshape(B, 768)

    # lna affine folds into W_dy (t_cond only feeds D_y):
    #   relu((u*g + b) @ Wdy) = relu(u @ (g[:,None]*Wdy) + b@Wdy)
    W_dy = np.ascontiguousarray(
        np.transpose(ap["D_y"], (1, 0, 2)).reshape(D, NTOT) * ap["lna_g"][:, None])
    c_dy = ap["lna_b"] @ np.transpose(ap["D_y"], (1, 0, 2)).reshape(D, NTOT)

    W_dx = np.ascontiguousarray(np.transpose(ap["D_x"], (1, 0, 2)).reshape(D, NTOT))

    # enc_ln affine folds into enc_w2.
    W_e2 = np.ascontiguousarray(ap["enc_w2"] * ap["enc_ln_g"][:, None])
    c_e2 = ap["enc_ln_b"] @ ap["enc_w2"] + ap["enc_b2"]

    def pmajor(w):
        # [K, F] -> [128, (K//128)*F]: partition p gets rows {kc*128+p}
        Kd, F = w.shape
        return w.reshape(Kd // 128, 128, F).transpose(1, 0, 2).reshape(128, -1)

    wblob = np.ascontiguousarray(np.concatenate(
        [pmajor(ap["enc_w1"]), pmajor(W_e2),
         pmajor(ap["By_w"]), pmajor(ap["Bx_w"]), pmajor(ap["xinit_w"]),
         pmajor(W_dy), pmajor(ap["E"]), pmajor(W_dx), pmajor(ap["vh_w1"])],
        axis=1))

    host = dict(
        lam=lam,
        boards=boards,
        wblob=wblob,
        vh_w2=np.ascontiguousarray(np.concatenate(
            [ap["vh_w2"], np.zeros((D // 2, 1), f32)], axis=1)),
        # possibly-zero bias vectors (skipped at trace time when all-zero)
        enc_b1=ap["enc_b1"],
        c_e2=c_e2.astype(f32),
        xinit_b=ap["xinit_b"],
        c_dy=c_dy.astype(f32),
        vh_b1=ap["vh_b1"],
        vh_b2=float(ap["vh_b2"].reshape(-1)[0]),
        # z-LN affine (identity in practice; applied for real if not)
        lnz_g=ap["lnz_g"],
        lnz_b=ap["lnz_b"],
    )
    return host


def _nz(v):
    return float(np.abs(v).max()) != 0.0


def _build(host):
    """Build the Bass program for one core (SPMD: all cores identical)."""
    import concourse.bass as bass
    import concourse.tile as tile
    from concourse import bacc, mybir
    from concourse.masks import make_identity

    F32 = mybir.dt.float32
    F32R = mybir.dt.float32r
    FP16 = mybir.dt.float16
    ALU = mybir.AluOpType
    ACTF = mybir.ActivationFunctionType
    GELU = ACTF.Tanh if SIM_GELU_SUB else ACTF.Gelu

    lam = host["lam"]
    use_enc_b1 = _nz(host["enc_b1"])
    use_c_e2 = _nz(host["c_e2"])
    use_xinit_b = _nz(host["xinit_b"])
    use_c_dy = _nz(host["c_dy"])
    use_vh_b1 = _nz(host["vh_b1"])
    use_vh_b2 = host["vh_b2"] != 0.0
    use_lnz_aff = _nz(host["lnz_g"] - 1.0) or _nz(host["lnz_b"])

    nc = bacc.Bacc(None, target_bir_lowering=False)

    # ---- DRAM I/O ----
    dr = {}
    def din(name, shape, dt=F32):
        dr[name] = nc.dram_tensor(name, list(shape), dt, kind="ExternalInput")
        return dr[name]

    WBLOB = (6 * D + 2 * D + 2 * (2 * D) + 3 * (2 * NTOT) + 16 * D
             + 2 * (D // 2))  # 19712
    din("boards", (BC, 768), FP16)
    din("wblob", (128, WBLOB), FP16)
    din("vh_w2", (D // 2, 2), FP16)
    for nm, sz in (("enc_b1", D), ("c_e2", D), ("xinit_b", NTOT), ("c_dy", NTOT),
                   ("vh_b1", D // 2)):
        if _nz(host[nm]):
            din(nm, (sz,))
    if use_lnz_aff:
        din("lnz_g", (D,))
        din("lnz_b", (D,))
    v_out = nc.dram_tensor("v_out", [BC, 1], F32, kind="ExternalOutput")

    with tile.TileContext(nc) as tc, ExitStack() as ctx:
        consts = ctx.enter_context(tc.tile_pool(name="consts", bufs=1))
        state = ctx.enter_context(tc.tile_pool(name="state", bufs=1))
        work = ctx.enter_context(tc.tile_pool(name="work", bufs=2))
        dump = ctx.enter_context(tc.tile_pool(name="dump", bufs=2))
        # PSUM: 8 banks x 2KB/partition; per-(pool,tag) ring of `bufs` buffers.
        # fm(2 banks) + gram(2) + mm(1) + tr(1) + lift(1) <= 8 banks.
        ps_fm = ctx.enter_context(tc.tile_pool(name="ps_fm", bufs=1, space="PSUM"))
        ps_gram = ctx.enter_context(tc.tile_pool(name="ps_gram", bufs=3, space="PSUM"))
        ps_mm = ctx.enter_context(tc.tile_pool(name="ps_mm", bufs=1, space="PSUM"))
        ps_tr = ctx.enter_context(tc.tile_pool(name="ps_tr", bufs=1, space="PSUM"))
        ps_lift = ctx.enter_context(tc.tile_pool(name="ps_lift", bufs=1, space="PSUM"))

        # ---- load weights to SBUF, in first-use order ----
        def wtile(name, kdim, fdim, dt=F32R):
            t = consts.tile([128, kdim // 128, fdim], dt, tag=f"w_{name}")
            nc.gpsimd.dma_start(out=t, in_=dr[name][:].rearrange(
                "(po p) f -> p po f", p=128))
            return t

        # consts (identities, eps) build on Pool FIRST so the PE can start
        # the encoder transposes while weights stream in
        identf = consts.tile([64, 64], F32, tag="identf")
        make_identity(nc, identf)
        identf128 = consts.tile([128, 128], F32, tag="identf128")
        make_identity(nc, identf128)
        identb = consts.tile([128, 128], FP16, tag="identb")
        make_identity(nc, identb)
        idb64 = identb[0:64, 0:64]
        idb64u = identb[64:128, 64:128]
        eps_t = consts.tile([BC, 1], F32, tag="eps")
        nc.vector.memset(eps_t, EPS_LN)
        vhb2_t = None
        if use_vh_b2:
            vhb2_t = consts.tile([BC, 1], F32, tag="vhb2")
            nc.vector.memset(vhb2_t, host["vh_b2"])

        boards_sb = work.tile([BC, 768], FP16, tag="boards")
        nc.sync.dma_start(out=boards_sb, in_=dr["boards"][:])

        wblob = consts.tile([128, WBLOB], FP16, tag="wblob")
        # chunked in first-use order; enc_w1 ships alone so the encoder
        # matmuls start as early as possible
        # (enc_w1 | W_e2/By/Bx | xinit | W_dy | E | W_dx/vh1)
        _c0 = 6 * D + 2 * D + 2 * (2 * D)
        _cuts = [0, 6 * D, _c0, _c0 + 2 * NTOT, _c0 + 4 * NTOT,
                 _c0 + 4 * NTOT + 16 * D, WBLOB]
        for ci in range(6):
            nc.gpsimd.dma_start(out=wblob[:, _cuts[ci]:_cuts[ci + 1]],
                                in_=dr["wblob"][:, _cuts[ci]:_cuts[ci + 1]])
        _off = [0]
        def wview(po, fdim):
            v = wblob[:, _off[0]:_off[0] + po * fdim].rearrange(
                "p (po f) -> p po f", po=po)
            _off[0] += po * fdim
            return v
        w_enc1 = wview(6, D)
        w_e2 = wview(2, D)
        w_by = wview(2, D)
        w_bx = wview(2, D)
        w_xinit = wview(2, NTOT)
        w_dy = wview(2, NTOT)
        w_e = wview(16, D)
        w_dx = wview(2, NTOT)
        w_vh1 = wview(2, D // 2)
        w_vh2 = consts.tile([128, 2], FP16, tag="w_vh2")
        nc.sync.dma_start(out=w_vh2, in_=dr["vh_w2"][:])

        def bias_bcast(name, fdim, parts=BC):
            t = consts.tile([parts, fdim], F32, tag=f"b_{name}")
            src = dr[name][:]
            nc.sync.dma_start(out=t, in_=bass.AP(
                tensor=src.tensor, offset=src.offset,
                ap=[[0, parts]] + list(src.ap)))
            return t

        def bias_bcast_T(name):
            """(NTOT,) bias -> T-form broadcast tile [128, 8, 128]."""
            t = consts.tile([128, 8, 128], F32, tag=f"bT_{name}")
            src = dr[name][:]
            for s in range(8):
                tt, cc = s // 4, s % 4
                for hh in range(2):
                    h = 2 * tt + hh
                    nc.sync.dma_start(out=t[:, s, hh * BC:(hh + 1) * BC],
                                      in_=bass.AP(
                        tensor=src.tensor,
                        offset=src.offset + h * NN + cc * 128,
                        ap=[[1, 128], [0, BC]]))
            return t

        b_enc1 = bias_bcast("enc_b1", D) if use_enc_b1 else None
        b_ce2 = bias_bcast("c_e2", D) if use_c_e2 else None
        b_vh1 = bias_bcast("vh_b1", D // 2) if use_vh_b1 else None
        b_xinit = bias_bcast_T("xinit_b") if use_xinit_b else None
        b_cdy = bias_bcast_T("c_dy") if use_c_dy else None
        g_lnz = bias_bcast("lnz_g", D) if use_lnz_aff else None
        b_lnz = bias_bcast("lnz_b", D) if use_lnz_aff else None

        # ---- persistent state ----
        # T-form x slots: [n=128, slot s=4t+cc, 64*hh+b]; head h = 2t+hh.
        xthist = state.tile([128, K, 8, 128], FP16, tag="xthist")
        z_hist = state.tile([128, K - 1, 2, DH], FP16, tag="z_hist")
        a_zbh = state.tile([128, 2, DH], FP16, tag="a_zbh")
        Sp = state.tile([128, 2 * (K - 1)], F32, tag="Sp")
        rw = state.tile([128, K - 1, 2], F32, tag="rw")          # c_m * recip_m
        sq = state.tile([128, 2], F32, tag="sq")                 # ||x_k||^2 per head
        b_y = state.tile([BC, D], F32, tag="b_y")
        b_x = state.tile([BC, D], FP16, tag="b_x")

        # ---------------- helpers ----------------
        def bm_to_fm(src_bm, nch, tag):
            """(64, nch*128) f32 BM -> (128, nch, 64) F32R FM via PE transpose."""
            ps = ps_tr.tile([128, max(nch, 2), 64], F32, tag="tr")
            for c in range(nch):
                nc.tensor.transpose(ps[:, c, :], src_bm[:, c * 128:(c + 1) * 128],
                                    identf)
            sb = work.tile([128, max(nch, 2), 64], F32R, tag=f"sbt_{tag}")
            nc.scalar.copy(out=sb[:, :nch, :], in_=ps[:, :nch, :])
            return sb

        def bm_to_fm_bf(src_bm, nch, tag):
            """(64, nch*128) fp16 BM -> (128, nch, 64) FP16 FM via PE transpose."""
            ps = ps_tr.tile([128, max(nch, 2), 64], FP16, tag="tr")
            for c in range(nch):
                nc.tensor.transpose(ps[:, c, :], src_bm[:, c * 128:(c + 1) * 128],
                                    idb64)
            sb = work.tile([128, max(nch, 2), 64], FP16, tag=f"sbt_{tag}")
            nc.scalar.copy(out=sb[:, :nch, :], in_=ps[:, :nch, :])
            return sb

        def mm(psum_out, lhsT_fm, w_sb, nch, n0, nfree):
            """psum_out (64, nfree) += x @ W[:, n0:n0+nfree] over nch k-chunks."""
            for kc in range(nch):
                nc.tensor.matmul(psum_out,
                                 lhsT_fm[:, kc, :],
                                 w_sb[:, kc, n0:n0 + nfree],
                                 start=(kc == 0), stop=(kc == nch - 1))

        def mm_fmT(ps_ab, act_fm, w_sb, nch):
            """(D->NTOT) matmul in FM orientation: stationary = weight chunk,
            moving = 64-wide fp16 activations; out T-form, split into two
            per-head-pair tiles so each half's consumer starts as soon as its
            16 matmuls finish (PSUM deps are whole-tile)."""
            for h in range(NH):
                for cc in range(4):
                    hh = h & 1
                    dst = ps_ab[h >> 1][:, cc, hh * BC:(hh + 1) * BC]
                    for kc in range(nch):
                        nc.tensor.matmul(
                            dst,
                            w_sb[:, kc, h * NN + cc * 128:h * NN + (cc + 1) * 128],
                            act_fm[:, kc, :], start=(kc == 0), stop=(kc == nch - 1))

        def gram(ps_out, m_slot, k_slot, t):
            """ps_out[128,128] = sum over head-pair-t chunks of
            xT[m][n, (hh,b)] x xT[k][n, (hh,b)'] — diag = per-(hh,b) dots."""
            for cc in range(4):
                nc.tensor.matmul(ps_out,
                                 xthist[:, m_slot, 4 * t + cc, :],
                                 xthist[:, k_slot, 4 * t + cc, :],
                                 start=(cc == 0), stop=(cc == 3))

        def layer_norm(dst_bm, src_ap, tag):
            """dst = (src - mean)/sqrt(var+eps) over last-dim 256 (BM layout)."""
            st6 = work.tile([BC, 6], F32, tag=f"ln6_{tag}")
            mv = work.tile([BC, 2], F32, tag=f"lnmv_{tag}")
            nc.vector.bn_stats(out=st6, in_=src_ap)
            nc.vector.bn_aggr(out=mv, in_=st6)
            rstd = work.tile([BC, 1], F32, tag=f"lnr_{tag}")
            nc.scalar.activation(out=rstd, in_=mv[:, 1:2], func=ACTF.Sqrt,
                                 bias=eps_t, scale=1.0)
            nc.vector.reciprocal(out=rstd, in_=rstd)
            nc.vector.tensor_scalar(out=dst_bm, in0=src_ap, scalar1=mv[:, 0:1],
                                    scalar2=rstd, op0=ALU.subtract, op1=ALU.mult)

        # ---------------- encoder ----------------
        bT = bm_to_fm_bf(boards_sb, 6, "bT")

        h_ps = ps_mm.tile([BC, D], F32, tag="mm")
        mm(h_ps, bT, w_enc1, 6, 0, D)
        h_sb = work.tile([BC, D], F32, tag="h_sb")
        if use_enc_b1:
            nc.vector.tensor_add(out=h_ps, in0=h_ps, in1=b_enc1)
        nc.scalar.activation(out=h_sb, in_=h_ps, func=GELU)

        hln = work.tile([BC, D], FP16, tag="hln")
        layer_norm(hln, h_sb, "enc")
        hlnT = bm_to_fm_bf(hln, 2, "s2b")

        be_ps = ps_mm.tile([BC, D], F32, tag="mm")
        mm(be_ps, hlnT, w_e2, 2, 0, D)
        benc = work.tile([BC, D], FP16, tag="benc")
        if use_c_e2:
            nc.vector.tensor_add(out=benc, in0=be_ps, in1=b_ce2)
        else:
            nc.scalar.copy(out=benc, in_=be_ps)
        bencT = bm_to_fm_bf(benc, 2, "s2b")

        by_ps = ps_mm.tile([BC, D], F32, tag="mm")
        mm(by_ps, bencT, w_by, 2, 0, D)
        nc.scalar.copy(out=b_y, in_=by_ps)
        bx_ps = ps_mm.tile([BC, D], F32, tag="mm")
        mm(bx_ps, bencT, w_bx, 2, 0, D)
        nc.scalar.copy(out=b_x, in_=bx_ps)

        x0_a = ps_fm.tile([128, 4, 128], F32, tag="fmA")
        x0_b = ps_fm.tile([128, 4, 128], F32, tag="fmB")
        x0_ab = (x0_a, x0_b)
        mm_fmT(x0_ab, bencT, w_xinit, 2)
        for t in range(2):
            if use_xinit_b:
                nc.vector.tensor_add(out=x0_ab[t], in0=x0_ab[t],
                                     in1=b_xinit[:, 4 * t:4 * t + 4, :])
            nc.scalar.activation(out=xthist[:, 0, 4 * t:4 * t + 4, :],
                                 in_=x0_ab[t], func=ACTF.Relu)

        # ---------------- K thinking steps ----------------
        z_bm = None
        for k in range(K):
            # -- S' dots as Gram matmuls; self-dot from cached ||x_k||^2 --
            if k >= 1:
                for m in range(1, k):
                    for t in range(2):
                        gp = ps_gram.tile([128, 128], F32, tag="gram")
                        gram(gp, m, k, t)
                        gd = dump.tile([128, 128], FP16, tag="gd")
                        nc.vector.scalar_tensor_tensor(
                            out=gd, in0=gp, scalar=rw[:, m - 1, t:t + 1],
                            in1=identb, op0=ALU.mult, op1=ALU.mult,
                            accum_out=Sp[:, 2 * (m - 1) + t:2 * (m - 1) + t + 1])
                for t in range(2):
                    nc.vector.tensor_scalar_mul(
                        out=Sp[:, 2 * (k - 1) + t:2 * (k - 1) + t + 1],
                        in0=sq[:, t:t + 1], scalar1=rw[:, k - 1, t:t + 1])
                for m in range(1, k + 1):
                    for t in range(2):
                        c = 2 * (m - 1) + t
                        if m == 1:
                            nc.vector.tensor_scalar_mul(
                                out=a_zbh[:, t, :], in0=z_hist[:, 0, t, :],
                                scalar1=Sp[:, c:c + 1])
                        else:
                            nc.vector.scalar_tensor_tensor(
                                out=a_zbh[:, t, :], in0=z_hist[:, m - 1, t, :],
                                scalar=Sp[:, c:c + 1], in1=a_zbh[:, t, :],
                                op0=ALU.mult, op1=ALU.add)
                # a (ZBH) -> BM fused with + b_y and the lam^k factor
                ps_a = ps_lift.tile([BC, 2, DH], F32, tag="lift")
                for t in range(2):
                    nc.tensor.matmul(ps_a[:, t, :], idb64u, a_zbh[64:128, t, :],
                                     start=True, stop=True)
                t_in = work.tile([BC, D], F32, tag="t_in")
                tv = t_in[:].rearrange("b (h d) -> b h d", h=NH)
                byv = b_y[:].rearrange("b (h d) -> b h d", h=NH)
                lamk = float(lam ** k)
                nc.vector.scalar_tensor_tensor(
                    out=tv[:, 1::2, :], in0=ps_a, scalar=lamk,
                    in1=byv[:, 1::2, :], op0=ALU.mult, op1=ALU.add)
                nc.vector.scalar_tensor_tensor(
                    out=tv[:, 0::2, :], in0=a_zbh[0:BC], scalar=lamk,
                    in1=byv[:, 0::2, :], op0=ALU.mult, op1=ALU.add)
            else:
                t_in = b_y

            # -- t_cond = LN(a + b_y)  (lna affine folded into W_dy) --
            tc_bm = work.tile([BC, D], FP16, tag="tc_bm")
            layer_norm(tc_bm, t_in[:], f"a{k}")
            tcT = bm_to_fm_bf(tc_bm, 2, "s2b")

            # -- g = relu(tc @ W_dy); y = g * x  (T-form, single STT) --
            g_a = ps_fm.tile([128, 4, 128], F32, tag="fmA")
            g_b = ps_fm.tile([128, 4, 128], F32, tag="fmB")
            g_ab = (g_a, g_b)
            mm_fmT(g_ab, tcT, w_dy, 2)
            y_t = work.tile([128, 8, 128], FP16, tag="y_t")
            for t in range(2):
                if use_c_dy:
                    nc.vector.tensor_add(out=g_ab[t], in0=g_ab[t],
                                         in1=b_cdy[:, 4 * t:4 * t + 4, :])
                nc.vector.scalar_tensor_tensor(
                    out=y_t[:, 4 * t:4 * t + 4, :],
                    in0=g_ab[t],
                    scalar=0.0, in1=xthist[:, k, 4 * t:4 * t + 4, :],
                    op0=ALU.max, op1=ALU.mult)

            # -- z = LN(y @ E): y_t chunks are the lhsT directly (BM out) --
            z_ps = ps_mm.tile([BC, D], F32, tag="mm")
            first = True
            for t in range(2):
                for hh in range(2):
                    h = 2 * t + hh
                    for cc in range(4):
                        nc.tensor.matmul(
                            z_ps, y_t[:, 4 * t + cc, hh * BC:(hh + 1) * BC],
                            w_e[:, h * 4 + cc, :],
                            start=first, stop=(t == 1 and hh == 1 and cc == 3))
                        first = False
            z_bm = work.tile([BC, D], FP16, tag="z_bm")
            layer_norm(z_bm, z_ps[:], f"z{k}")
            if use_lnz_aff:
                nc.vector.tensor_mul(out=z_bm, in0=z_bm, in1=g_lnz)
                nc.vector.tensor_add(out=z_bm, in0=z_bm, in1=b_lnz)

            if k == K - 1:
                break  # x update not needed after the last step

            # -- store z into ZBH history via PE identity-lift --
            zv = z_bm[:].rearrange("b (h d) -> b h d", h=NH)
            zl_ps = ps_lift.tile([128, 2, DH], F32, tag="lift")
            for t in range(2):
                nc.tensor.matmul(zl_ps[0:BC, t, :], idb64, zv[:, 2 * t, :],
                                 start=True, stop=True)
                nc.tensor.matmul(zl_ps[BC:128, t, :], idb64, zv[:, 2 * t + 1, :],
                                 start=True, stop=True)
            nc.scalar.copy(out=z_hist[:, k], in_=zl_ps)

            # -- dx = relu((z + b_x) @ W_dx); x_new = x + dx (T-form) --
            zbx = work.tile([BC, D], FP16, tag="zbx")
            nc.vector.tensor_add(out=zbx, in0=z_bm, in1=b_x)
            zbxT = bm_to_fm_bf(zbx, 2, "s2b")
            dx_a = ps_fm.tile([128, 4, 128], F32, tag="fmA")
            dx_b = ps_fm.tile([128, 4, 128], F32, tag="fmB")
            dx_ab = (dx_a, dx_b)
            mm_fmT(dx_ab, zbxT, w_dx, 2)
            for t in range(2):
                nc.vector.scalar_tensor_tensor(
                    out=xthist[:, k + 1, 4 * t:4 * t + 4, :],
                    in0=dx_ab[t], scalar=0.0,
                    in1=xthist[:, k, 4 * t:4 * t + 4, :],
                    op0=ALU.max, op1=ALU.add)

            # -- ||x_{k+1}||^2 via self-Gram diag; rw[k] = c_{k+1}/(||x||+eps) --
            for t in range(2):
                gp = ps_gram.tile([128, 128], F32, tag="gram")
                gram(gp, k + 1, k + 1, t)
                gd = dump.tile([128, 128], FP16, tag="gd")
                nc.vector.scalar_tensor_tensor(
                    out=gd, in0=gp, scalar=1.0, in1=identb,
                    op0=ALU.mult, op1=ALU.mult, accum_out=sq[:, t:t + 1])
            nrm = work.tile([128, 2], F32, tag="nrm")
            nc.scalar.activation(out=nrm, in_=sq, func=ACTF.Sqrt)
            nc.vector.tensor_scalar_add(out=nrm, in0=nrm, scalar1=EPS_NRM)
            nc.vector.reciprocal(out=nrm, in_=nrm)
            c_m = float((1.0 - lam) * lam ** (-(k + 1)))
            nc.vector.tensor_scalar_mul(out=rw[:, k, :], in0=nrm, scalar1=c_m)

        # ---------------- value head ----------------
        zT = bm_to_fm_bf(z_bm, 2, "s2b")
        v1_ps = ps_mm.tile([BC, D // 2], F32, tag="mm")
        mm(v1_ps, zT, w_vh1, 2, 0, D // 2)
        if use_vh_b1:
            nc.vector.tensor_add(out=v1_ps, in0=v1_ps, in1=b_vh1)
        gl = work.tile([BC, D // 2], FP16, tag="gl")
        nc.scalar.activation(out=gl, in_=v1_ps, func=GELU)
        glT = bm_to_fm_bf(gl, 1, "s1")
        v_ps = ps_mm.tile([BC, 2], F32, tag="mm")
        nc.tensor.matmul(v_ps, glT[:, 0, :], w_vh2, start=True, stop=True)
        v_sb = work.tile([BC, 1], F32, tag="v_sb")
        if use_vh_b2:
            nc.scalar.activation(out=v_sb, in_=v_ps[:, 0:1], func=ACTF.Tanh,
                                 bias=vhb2_t)
        else:
            nc.scalar.activation(out=v_sb, in_=v_ps[:, 0:1], func=ACTF.Tanh)
        nc.sync.dma_start(out=v_out[:], in_=v_sb)

    nc.compile()
    return nc


_CACHE = {}


def kernel(**inputs):
    host = _prep_host(inputs)
    nc = _build(host)

    common = {k: v for k, v in host.items()
              if isinstance(v, np.ndarray) and k != "boards"}
    fp16_names = {"wblob", "vh_w2"}
    declared = {m.memorylocations[0].name
                for m in nc.m.functions[0].allocations
                if getattr(m, "kind", None) == "ExternalInput"}
    in_maps = []
    for c in range(NCORES):
        m = {"boards": np.ascontiguousarray(
            host["boards"][c * BC:(c + 1) * BC], dtype=np.float16)}
        for k, v in common.items():
            if k in declared:
                m[k] = (np.ascontiguousarray(v, dtype=np.float16)
                        if k in fp16_names else v)
        in_maps.append(m)

    from concourse.bass_utils import run_bass_kernel_spmd
    res = run_bass_kernel_spmd(nc, in_maps, core_ids=list(range(NCORES)))
    out = np.concatenate([res.results[c]["v_out"] for c in range(NCORES)], axis=0)
    return out.astype(np.float32)
